# revision 18
# baseline (speedup 1.0000x reference)
"""Causal self-attention on 8 trn2 NeuronCores.

Sharding: core c handles batch b = c//4 and heads 4*(c%4) .. 4*(c%4)+3
(data parallel on B, tensor parallel on the 16 heads). Each core computes
its 4 heads' attention plus the corresponding slice of the output
projection; the host sums the 4 partial projections per batch and adds bo.

On-chip layout is feature-major ("transposed"): qT/kT are [head_dim, seq],
scores are computed as sT[k, q] so the attention@v matmul needs no
transposes. The softmax denominator comes from an extra all-ones column
appended to Wv (so ctx PSUM row 64 accumulates sum_k exp). Normalization
broadcasts 1/denom across partitions via a DRAM bounce.

All matmuls run in float32r (TF32-like fp32 mode, 4x faster than fp32,
measured ~1.5e-4 L2 error per matmul).
"""

import sys

sys.path.insert(0, "/opt/trn_rl_repo")

import numpy as np

import concourse.bass as bass
import concourse.tile as tile
from concourse import bacc, mybir
from concourse.bass_utils import run_bass_kernel_spmd

B, S, D, H = 2, 2048, 1024, 16
HD = D // H            # 64
NCORES = 8
HPC = 4                # heads per core
DPC = HPC * HD         # 256 feature dims per core
QT = 512               # q tile (free dim of score matmuls)
KC = 128               # k chunk (partition dim of transposed scores)
NQT = S // QT          # 4
NKC = S // KC          # 16
VW = HPC * (HD + 1)    # 260: v with ones column per head

F32 = mybir.dt.float32
F32R = mybir.dt.float32r

_cache = {}


def _build(blocks, n_pat):
    """blocks: per q-tile, tuple of (ki, pat_idx|None) chunks to compute."""
    nc = bacc.Bacc(
        "TRN2",
        target_bir_lowering=False,
        debug=False,
        enable_asserts=False,
        num_devices=NCORES,
    )

    xt_d = nc.dram_tensor("xt", [D, S], F32R, kind="ExternalInput").ap()
    wq_d = nc.dram_tensor("wq", [D, DPC], F32R, kind="ExternalInput").ap()
    wk_d = nc.dram_tensor("wk", [D, DPC], F32R, kind="ExternalInput").ap()
    wv_d = nc.dram_tensor("wv", [D, VW], F32R, kind="ExternalInput").ap()
    wo_d = nc.dram_tensor("wo", [DPC, D], F32R, kind="ExternalInput").ap()
    bq_d = nc.dram_tensor("bq", [128, 2], F32, kind="ExternalInput").ap()
    bk_d = nc.dram_tensor("bk", [128, 2], F32, kind="ExternalInput").ap()
    bv_d = nc.dram_tensor("bv", [VW], F32, kind="ExternalInput").ap()
    mp_d = nc.dram_tensor("mp", [128, max(n_pat, 1) * QT], F32, kind="ExternalInput").ap()
    out_d = nc.dram_tensor("out", [S, D], F32, kind="ExternalOutput").ap()

    with tile.TileContext(nc) as tc:
        with (
            tc.tile_pool(name="consts", bufs=1) as consts,
            tc.tile_pool(name="mm_ps", bufs=2, space="PSUM") as mm_ps,
            tc.tile_pool(name="st_ps", bufs=2, space="PSUM") as st_ps,
            tc.tile_pool(name="ctx_ps", bufs=2, space="PSUM") as ctx_ps,
            tc.tile_pool(name="op_ps", bufs=2, space="PSUM") as op_ps,
            tc.tile_pool(name="work", bufs=4) as work,
            tc.tile_pool(name="norm", bufs=2) as norm,
            tc.tile_pool(name="ctxn", bufs=2) as ctxn,
            tc.tile_pool(name="stage", bufs=3) as stage,
        ):
            # ---- resident loads (ordered so PE can start after wq + x block 0) ----
            wq_sb = consts.tile([128, 8, DPC], F32R)
            for kc in range(8):
                nc.sync.dma_start(out=wq_sb[:, kc, :], in_=wq_d[kc * 128:(kc + 1) * 128, :])
            bq_sb = consts.tile([128, 2], F32)
            bk_sb = consts.tile([128, 2], F32)
            nc.sync.dma_start(out=bq_sb, in_=bq_d)
            nc.sync.dma_start(out=bk_sb, in_=bk_d)
            # x^T split into 4 sequence blocks of 512 so compute starts early
            xtb = []
            for nb in range(NQT):
                xtb_t = consts.tile([128, 8, QT], F32R, tag=f"xtb{nb}")
                xtb.append(xtb_t)
            for kc in range(8):
                nc.sync.dma_start(
                    out=xtb[0][:, kc, :], in_=xt_d[kc * 128:(kc + 1) * 128, 0:QT]
                )
            wk_sb = consts.tile([128, 8, DPC], F32R)
            wv_sb = consts.tile([128, 8, VW], F32R)
            for kc in range(8):
                nc.sync.dma_start(out=wk_sb[:, kc, :], in_=wk_d[kc * 128:(kc + 1) * 128, :])
            for kc in range(8):
                nc.sync.dma_start(out=wv_sb[:, kc, :], in_=wv_d[kc * 128:(kc + 1) * 128, :])
            bv_sb = consts.tile([128, VW], F32)
            nc.sync.dma_start(
                out=bv_sb,
                in_=bass.AP(tensor=bv_d.tensor, offset=0, ap=[[0, 128], [1, VW]]),
            )
            for nb in range(1, NQT):
                for kc in range(8):
                    nc.sync.dma_start(
                        out=xtb[nb][:, kc, :],
                        in_=xt_d[kc * 128:(kc + 1) * 128, nb * QT:(nb + 1) * QT],
                    )
            mp_sb = consts.tile([128, max(n_pat, 1), QT], F32)
            for p in range(max(n_pat, 1)):
                nc.sync.dma_start(out=mp_sb[:, p, :], in_=mp_d[:, p * QT:(p + 1) * QT])
            wo_sb = consts.tile([128, 2, D], F32R)
            for cc in range(2):
                nc.sync.dma_start(out=wo_sb[:, cc, :], in_=wo_d[cc * 128:(cc + 1) * 128, :])

            ones_f = consts.tile([65, HD], F32)
            nc.vector.memset(ones_f, 1.0)
            ones_r = consts.tile([65, HD], F32R)
            nc.vector.tensor_copy(ones_r, ones_f)

            # ---- phase A: qT/kT = W @ xT, v = x @ Wv_aug (feature-major q/k) ----
            qt_sb = consts.tile([128, 2, S], F32R)
            kt_sb = consts.tile([128, 2, S], F32R)
            v_sb = consts.tile([128, NKC, VW], F32R)

            def phase_a_block(n):
                for m in range(2):
                    ps = mm_ps.tile([128, QT], F32, tag="mm")
                    for kc in range(8):
                        nc.tensor.matmul(
                            ps,
                            wq_sb[:, kc, m * 128:(m + 1) * 128],
                            xtb[n][:, kc, :],
                            start=(kc == 0), stop=(kc == 7),
                        )
                    nc.vector.tensor_scalar_add(
                        qt_sb[:, m, n * QT:(n + 1) * QT], ps, bq_sb[:, m:m + 1]
                    )
                    ps = mm_ps.tile([128, QT], F32, tag="mm")
                    for kc in range(8):
                        nc.tensor.matmul(
                            ps,
                            wk_sb[:, kc, m * 128:(m + 1) * 128],
                            xtb[n][:, kc, :],
                            start=(kc == 0), stop=(kc == 7),
                        )
                    nc.vector.tensor_scalar_add(
                        kt_sb[:, m, n * QT:(n + 1) * QT], ps, bk_sb[:, m:m + 1]
                    )
                for sc in range(4 * n, 4 * n + 4):
                    ps = mm_ps.tile([128, VW], F32, tag="mm")
                    for kc in range(8):
                        nc.tensor.matmul(
                            ps,
                            xtb[n][:, kc, (sc % 4) * 128:(sc % 4 + 1) * 128],
                            wv_sb[:, kc, :],
                            start=(kc == 0), stop=(kc == 7),
                        )
                    nc.vector.tensor_add(v_sb[:, sc, :], ps, bv_sb)

            # ---- phase B/C: attention + output projection per q tile ----
            def attention_qtile(qi):
                qsl = slice(qi * QT, (qi + 1) * QT)
                cn0 = ctxn.tile([128, QT], F32R, tag="cn0")
                cn1 = ctxn.tile([128, QT], F32R, tag="cn1")
                cn = [cn0, cn1]

                for h in (2, 3, 0, 1):
                    even = (h % 2 == 0)
                    mc = h // 2                    # feature chunk of this head
                    fo = (h % 2) * HD              # feature offset within chunk
                    chunks = blocks[qi]
                    ctx = ctx_ps.tile([HD + 1, QT], F32)
                    for i, (ki, pat) in enumerate(chunks):
                        st = st_ps.tile([128, QT], F32)
                        nc.tensor.matmul(
                            st,
                            kt_sb[fo:fo + HD, mc, ki * 128:(ki + 1) * 128],
                            qt_sb[fo:fo + HD, mc, qsl],
                            start=True, stop=True,
                        )
                        ex = work.tile([128, QT], F32R)
                        nc.scalar.activation(
                            out=ex, in_=st,
                            func=mybir.ActivationFunctionType.Exp, scale=0.125,
                        )
                        if pat is not None:
                            nc.vector.tensor_mul(ex, ex, mp_sb[:, pat, :])
                        nc.tensor.matmul(
                            ctx,
                            v_sb[:, ki, h * (HD + 1):(h + 1) * (HD + 1)],
                            ex,
                            start=(i == 0), stop=(i == len(chunks) - 1),
                        )
                    # 1/denominator, broadcast across 64 partitions via a
                    # K=1 matmul (ones x recip) -- stays on-chip, no DMA hop.
                    # DVE reads only one PSUM operand, so evacuate ctx to SBUF
                    # first (also frees its PSUM bank early), then multiply
                    # against the broadcast left in PSUM.
                    dn_sb = norm.tile([HD + 1, QT], F32R, tag="dn")
                    with nc.allow_low_precision(reason="f32r operand for bcast matmul"):
                        nc.vector.reciprocal(dn_sb[HD:HD + 1, :], ctx[HD:HD + 1, :])
                    bc = op_ps.tile([HD, QT], F32, tag="op")
                    nc.tensor.matmul(
                        bc, ones_r[HD:HD + 1, :], dn_sb[HD:HD + 1, :],
                        start=True, stop=True,
                    )
                    tmp = norm.tile([HD, QT], F32R, tag="tmp")
                    with nc.allow_low_precision(reason="f32r ctx evacuation"):
                        nc.vector.tensor_copy(tmp, ctx[0:HD, :])
                    if even:
                        nc.vector.tensor_mul(cn[mc][0:HD, :], tmp, bc)
                    else:
                        tmp2 = norm.tile([HD, QT], F32R, tag="tmp2")
                        nc.vector.tensor_mul(tmp2, tmp, bc)
                        nc.sync.dma_start(out=cn[mc][HD:2 * HD, :], in_=tmp2)
                # output projection for this q tile
                for qc in range(4):
                    for ne in range(2):
                        ps = op_ps.tile([128, QT], F32, tag="op")
                        for cc in (1, 0):
                            nc.tensor.matmul(
                                ps,
                                cn[cc][:, qc * 128:(qc + 1) * 128],
                                wo_sb[:, cc, ne * QT:(ne + 1) * QT],
                                start=(cc == 1), stop=(cc == 0),
                            )
                        so = stage.tile([128, QT], F32)
                        nc.vector.tensor_copy(so, ps)
                        nc.sync.dma_start(
                            out=out_d[qi * QT + qc * 128: qi * QT + (qc + 1) * 128,
                                      ne * QT:(ne + 1) * QT],
                            in_=so,
                        )

            # interleave: emit each attention q-tile right after the phase-A
            # block that completes its inputs (block index = max ki // 4)
            ready_at = [max(ki for ki, _ in blocks[qi]) // 4 for qi in range(NQT)]
            for n in range(NQT):
                phase_a_block(n)
                for qi in range(NQT):
                    if ready_at[qi] == n:
                        attention_qtile(qi)

    nc.compile()
    return nc


def _block_structure(mask):
    """Classify [QT x KC] score blocks from the runtime mask (mask[q, k])."""
    allowed = ~np.isneginf(np.asarray(mask, dtype=np.float32))
    pats = []
    pat_idx = {}
    blocks = []
    for qi in range(NQT):
        row = []
        for ki in range(NKC):
            sub = allowed[qi * QT:(qi + 1) * QT, ki * KC:(ki + 1) * KC]
            if not sub.any():
                continue
            if sub.all():
                row.append((ki, None))
            else:
                pat = np.ascontiguousarray(sub.T.astype(np.float32))  # [128, 512]
                key = pat.tobytes()
                if key not in pat_idx:
                    pat_idx[key] = len(pats)
                    pats.append(pat)
                row.append((ki, pat_idx[key]))
        blocks.append(tuple(row))
    return tuple(blocks), pats


def kernel(x, mask, Wq, bq, Wk, bk, Wv, bv, Wo, bo):
    x = np.asarray(x, dtype=np.float32)
    blocks, pats = _block_structure(mask)
    n_pat = len(pats)
    key = (blocks, n_pat)
    if key not in _cache:
        _cache[key] = _build(blocks, n_pat)
    nc = _cache[key]

    if n_pat:
        mp = np.concatenate(pats, axis=1)          # [128, n_pat*QT]
    else:
        mp = np.zeros((128, QT), dtype=np.float32)

    xt = [np.ascontiguousarray(x[b].T) for b in range(B)]
    in_maps = []
    for c in range(NCORES):
        b, hg = c // HPC, c % HPC
        hs = slice(hg * DPC, (hg + 1) * DPC)
        wv_aug = np.zeros((D, VW), dtype=np.float32)
        bv_aug = np.zeros(VW, dtype=np.float32)
        for j in range(HPC):
            base = j * (HD + 1)
            rows = slice(hg * DPC + j * HD, hg * DPC + (j + 1) * HD)
            wv_aug[:, base:base + HD] = np.asarray(Wv)[rows, :].T
            bv_aug[base:base + HD] = np.asarray(bv)[rows]
            bv_aug[base + HD] = 1.0
        in_maps.append({
            "xt": xt[b],
            "wq": np.ascontiguousarray(np.asarray(Wq)[hs, :].T),
            "wk": np.ascontiguousarray(np.asarray(Wk)[hs, :].T),
            "wv": wv_aug,
            "wo": np.ascontiguousarray(np.asarray(Wo)[:, hs].T),
            "bq": np.ascontiguousarray(np.asarray(bq)[hs].reshape(2, 128).T),
            "bk": np.ascontiguousarray(np.asarray(bk)[hs].reshape(2, 128).T),
            "bv": bv_aug,
            "mp": mp,
        })

    res = run_bass_kernel_spmd(nc, in_maps, core_ids=list(range(NCORES))).results
    out = np.empty((B, S, D), dtype=np.float32)
    for b in range(B):
        acc = res[b * HPC]["out"].astype(np.float32).copy()
        for g in range(1, HPC):
            acc += res[b * HPC + g]["out"]
        out[b] = acc + np.asarray(bo, dtype=np.float32)[None, :]
    return out


# revision 19
# speedup vs baseline: 1.1840x; 1.1840x over previous
"""Causal self-attention on 8 trn2 NeuronCores.

Sharding: core c handles batch b = c//4 and heads 4*(c%4) .. 4*(c%4)+3
(data parallel on B, tensor parallel on the 16 heads). Each core computes
its 4 heads' attention plus the corresponding slice of the output
projection; the host sums the 4 partial projections per batch and adds bo.

On-chip layout is feature-major ("transposed"): qT/kT are [head_dim, seq],
scores are computed as sT[k, q] so the attention@v matmul needs no
transposes. The softmax denominator comes from an extra all-ones column
appended to Wv (so ctx PSUM row 64 accumulates sum_k exp). Normalization
broadcasts 1/denom across partitions via a DRAM bounce.

All matmuls run in float32r (TF32-like fp32 mode, 4x faster than fp32,
measured ~1.5e-4 L2 error per matmul).
"""

import sys

sys.path.insert(0, "/opt/trn_rl_repo")

import numpy as np

import concourse.bass as bass
import concourse.tile as tile
from concourse import bacc, mybir
from concourse.bass_utils import run_bass_kernel_spmd

B, S, D, H = 2, 2048, 1024, 16
HD = D // H            # 64
NCORES = 8
HPC = 4                # heads per core
DPC = HPC * HD         # 256 feature dims per core
QT = 512               # q tile (free dim of score matmuls)
KC = 128               # k chunk (partition dim of transposed scores)
NQT = S // QT          # 4
NKC = S // KC          # 16
VW = HPC * (HD + 1)    # 260: v with ones column per head

F32 = mybir.dt.float32
F32R = mybir.dt.float32r

_cache = {}


def _build(blocks, n_pat):
    """blocks: per q-tile, tuple of (ki, pat_idx|None) chunks to compute."""
    nc = bacc.Bacc(
        "TRN2",
        target_bir_lowering=False,
        debug=False,
        enable_asserts=False,
        num_devices=NCORES,
    )

    xt_d = nc.dram_tensor("xt", [D, S], F32R, kind="ExternalInput").ap()
    wq_d = nc.dram_tensor("wq", [D, DPC], F32R, kind="ExternalInput").ap()
    wk_d = nc.dram_tensor("wk", [D, DPC], F32R, kind="ExternalInput").ap()
    wv_d = nc.dram_tensor("wv", [D, VW], F32R, kind="ExternalInput").ap()
    wo_d = nc.dram_tensor("wo", [DPC, D], F32R, kind="ExternalInput").ap()
    bq_d = nc.dram_tensor("bq", [128, 2], F32, kind="ExternalInput").ap()
    bk_d = nc.dram_tensor("bk", [128, 2], F32, kind="ExternalInput").ap()
    bv_d = nc.dram_tensor("bv", [VW], F32, kind="ExternalInput").ap()
    mp_d = nc.dram_tensor("mp", [128, max(n_pat, 1) * QT], F32, kind="ExternalInput").ap()
    out_d = nc.dram_tensor("out", [S, D], F32, kind="ExternalOutput").ap()

    with tile.TileContext(nc) as tc:
        with (
            tc.tile_pool(name="consts", bufs=1) as consts,
            tc.tile_pool(name="mm_ps", bufs=2, space="PSUM") as mm_ps,
            tc.tile_pool(name="st_ps", bufs=2, space="PSUM") as st_ps,
            tc.tile_pool(name="ctx_ps", bufs=2, space="PSUM") as ctx_ps,
            tc.tile_pool(name="op_ps", bufs=2, space="PSUM") as op_ps,
            tc.tile_pool(name="work", bufs=4) as work,
            tc.tile_pool(name="norm", bufs=2) as norm,
            tc.tile_pool(name="ctxn", bufs=2) as ctxn,
            tc.tile_pool(name="stage", bufs=3) as stage,
        ):
            # ---- resident loads (ordered so PE can start after wq + x block 0) ----
            wq_sb = consts.tile([128, 8, DPC], F32R)
            for kc in range(8):
                nc.sync.dma_start(out=wq_sb[:, kc, :], in_=wq_d[kc * 128:(kc + 1) * 128, :])
            bq_sb = consts.tile([128, 2], F32)
            bk_sb = consts.tile([128, 2], F32)
            nc.sync.dma_start(out=bq_sb, in_=bq_d)
            nc.sync.dma_start(out=bk_sb, in_=bk_d)
            # x^T split into 4 sequence blocks of 512 so compute starts early
            xtb = []
            for nb in range(NQT):
                xtb_t = consts.tile([128, 8, QT], F32R, tag=f"xtb{nb}")
                xtb.append(xtb_t)
            for kc in range(8):
                nc.sync.dma_start(
                    out=xtb[0][:, kc, :], in_=xt_d[kc * 128:(kc + 1) * 128, 0:QT]
                )
            wk_sb = consts.tile([128, 8, DPC], F32R)
            wv_sb = consts.tile([128, 8, VW], F32R)
            for kc in range(8):
                nc.sync.dma_start(out=wk_sb[:, kc, :], in_=wk_d[kc * 128:(kc + 1) * 128, :])
            for kc in range(8):
                nc.sync.dma_start(out=wv_sb[:, kc, :], in_=wv_d[kc * 128:(kc + 1) * 128, :])
            bv_sb = consts.tile([128, VW], F32)
            nc.sync.dma_start(
                out=bv_sb,
                in_=bass.AP(tensor=bv_d.tensor, offset=0, ap=[[0, 128], [1, VW]]),
            )
            for nb in range(1, NQT):
                for kc in range(8):
                    nc.sync.dma_start(
                        out=xtb[nb][:, kc, :],
                        in_=xt_d[kc * 128:(kc + 1) * 128, nb * QT:(nb + 1) * QT],
                    )
            mp_sb = consts.tile([128, max(n_pat, 1), QT], F32)
            for p in range(max(n_pat, 1)):
                nc.sync.dma_start(out=mp_sb[:, p, :], in_=mp_d[:, p * QT:(p + 1) * QT])
            wo_sb = consts.tile([128, 2, D], F32R)
            for cc in range(2):
                nc.sync.dma_start(out=wo_sb[:, cc, :], in_=wo_d[cc * 128:(cc + 1) * 128, :])

            ones_f = consts.tile([65, HD], F32)
            nc.vector.memset(ones_f, 1.0)
            ones_r = consts.tile([65, HD], F32R)
            nc.vector.tensor_copy(ones_r, ones_f)

            # ---- phase A: qT/kT = W @ xT, v = x @ Wv_aug (feature-major q/k) ----
            qt_sb = consts.tile([128, 2, S], F32R)
            kt_sb = consts.tile([128, 2, S], F32R)
            v_sb = consts.tile([128, NKC, VW], F32R)

            def phase_a_block(n):
                for m in range(2):
                    ps = mm_ps.tile([128, QT], F32, tag="mm")
                    for kc in range(8):
                        nc.tensor.matmul(
                            ps,
                            wq_sb[:, kc, m * 128:(m + 1) * 128],
                            xtb[n][:, kc, :],
                            start=(kc == 0), stop=(kc == 7),
                        )
                    nc.vector.tensor_scalar_add(
                        qt_sb[:, m, n * QT:(n + 1) * QT], ps, bq_sb[:, m:m + 1]
                    )
                    ps = mm_ps.tile([128, QT], F32, tag="mm")
                    for kc in range(8):
                        nc.tensor.matmul(
                            ps,
                            wk_sb[:, kc, m * 128:(m + 1) * 128],
                            xtb[n][:, kc, :],
                            start=(kc == 0), stop=(kc == 7),
                        )
                    nc.vector.tensor_scalar_add(
                        kt_sb[:, m, n * QT:(n + 1) * QT], ps, bk_sb[:, m:m + 1]
                    )
                for sc in range(4 * n, 4 * n + 4):
                    ps = mm_ps.tile([128, VW], F32, tag="mm")
                    for kc in range(8):
                        nc.tensor.matmul(
                            ps,
                            xtb[n][:, kc, (sc % 4) * 128:(sc % 4 + 1) * 128],
                            wv_sb[:, kc, :],
                            start=(kc == 0), stop=(kc == 7),
                        )
                    nc.vector.tensor_add(v_sb[:, sc, :], ps, bv_sb)

            # ---- phase B/C: attention + output projection per q tile ----
            def attention_qtile(qi):
                qsl = slice(qi * QT, (qi + 1) * QT)
                cn0 = ctxn.tile([128, QT], F32R, tag="cn0")
                cn1 = ctxn.tile([128, QT], F32R, tag="cn1")
                cn = [cn0, cn1]

                for h in (2, 3, 0, 1):
                    even = (h % 2 == 0)
                    mc = h // 2                    # feature chunk of this head
                    fo = (h % 2) * HD              # feature offset within chunk
                    chunks = blocks[qi]
                    ctx = ctx_ps.tile([HD + 1, QT], F32)
                    for i, (ki, pat) in enumerate(chunks):
                        st = st_ps.tile([128, QT], F32)
                        nc.tensor.matmul(
                            st,
                            kt_sb[fo:fo + HD, mc, ki * 128:(ki + 1) * 128],
                            qt_sb[fo:fo + HD, mc, qsl],
                            start=True, stop=True,
                        )
                        ex = work.tile([128, QT], F32R)
                        nc.scalar.activation(
                            out=ex, in_=st,
                            func=mybir.ActivationFunctionType.Exp, scale=0.125,
                        )
                        if pat is not None:
                            nc.vector.tensor_mul(ex, ex, mp_sb[:, pat, :])
                        nc.tensor.matmul(
                            ctx,
                            v_sb[:, ki, h * (HD + 1):(h + 1) * (HD + 1)],
                            ex,
                            start=(i == 0), stop=(i == len(chunks) - 1),
                        )
                    # Normalize: broadcast the denominator across the 64 ctx
                    # partitions with a K=1 matmul (ones x denom), take the
                    # reciprocal with the fast Newton DVE op (which also
                    # evacuates the broadcast out of PSUM), then multiply the
                    # ctx rows (still in PSUM) by it on the way to SBUF.
                    dn_sb = norm.tile([HD + 1, QT], F32R, tag="dn")
                    with nc.allow_low_precision(reason="f32r operand for bcast matmul"):
                        nc.vector.tensor_copy(dn_sb[HD:HD + 1, :], ctx[HD:HD + 1, :])
                    bc = op_ps.tile([HD, QT], F32, tag="op")
                    nc.tensor.matmul(
                        bc, ones_r[HD:HD + 1, :], dn_sb[HD:HD + 1, :],
                        start=True, stop=True,
                    )
                    rc = norm.tile([HD, QT], F32, tag="rc")
                    nc.vector.reciprocal_approx_fast(out=rc, in_=bc)
                    if even:
                        nc.vector.tensor_mul(cn[mc][0:HD, :], ctx[0:HD, :], rc)
                    else:
                        tmp2 = norm.tile([HD, QT], F32R, tag="tmp2")
                        nc.vector.tensor_mul(tmp2, ctx[0:HD, :], rc)
                        nc.sync.dma_start(out=cn[mc][HD:2 * HD, :], in_=tmp2)
                # output projection for this q tile
                for qc in range(4):
                    for ne in range(2):
                        ps = op_ps.tile([128, QT], F32, tag="op")
                        for cc in (1, 0):
                            nc.tensor.matmul(
                                ps,
                                cn[cc][:, qc * 128:(qc + 1) * 128],
                                wo_sb[:, cc, ne * QT:(ne + 1) * QT],
                                start=(cc == 1), stop=(cc == 0),
                            )
                        so = stage.tile([128, QT], F32)
                        nc.vector.tensor_copy(so, ps)
                        nc.sync.dma_start(
                            out=out_d[qi * QT + qc * 128: qi * QT + (qc + 1) * 128,
                                      ne * QT:(ne + 1) * QT],
                            in_=so,
                        )

            # interleave: emit each attention q-tile right after the phase-A
            # block that completes its inputs (block index = max ki // 4)
            ready_at = [max(ki for ki, _ in blocks[qi]) // 4 for qi in range(NQT)]
            for n in range(NQT):
                phase_a_block(n)
                for qi in range(NQT):
                    if ready_at[qi] == n:
                        attention_qtile(qi)

    nc.compile()
    return nc


def _block_structure(mask):
    """Classify [QT x KC] score blocks from the runtime mask (mask[q, k])."""
    allowed = ~np.isneginf(np.asarray(mask, dtype=np.float32))
    pats = []
    pat_idx = {}
    blocks = []
    for qi in range(NQT):
        row = []
        for ki in range(NKC):
            sub = allowed[qi * QT:(qi + 1) * QT, ki * KC:(ki + 1) * KC]
            if not sub.any():
                continue
            if sub.all():
                row.append((ki, None))
            else:
                pat = np.ascontiguousarray(sub.T.astype(np.float32))  # [128, 512]
                key = pat.tobytes()
                if key not in pat_idx:
                    pat_idx[key] = len(pats)
                    pats.append(pat)
                row.append((ki, pat_idx[key]))
        blocks.append(tuple(row))
    return tuple(blocks), pats


def kernel(x, mask, Wq, bq, Wk, bk, Wv, bv, Wo, bo):
    x = np.asarray(x, dtype=np.float32)
    blocks, pats = _block_structure(mask)
    n_pat = len(pats)
    key = (blocks, n_pat)
    if key not in _cache:
        _cache[key] = _build(blocks, n_pat)
    nc = _cache[key]

    if n_pat:
        mp = np.concatenate(pats, axis=1)          # [128, n_pat*QT]
    else:
        mp = np.zeros((128, QT), dtype=np.float32)

    xt = [np.ascontiguousarray(x[b].T) for b in range(B)]
    in_maps = []
    for c in range(NCORES):
        b, hg = c // HPC, c % HPC
        hs = slice(hg * DPC, (hg + 1) * DPC)
        wv_aug = np.zeros((D, VW), dtype=np.float32)
        bv_aug = np.zeros(VW, dtype=np.float32)
        for j in range(HPC):
            base = j * (HD + 1)
            rows = slice(hg * DPC + j * HD, hg * DPC + (j + 1) * HD)
            wv_aug[:, base:base + HD] = np.asarray(Wv)[rows, :].T
            bv_aug[base:base + HD] = np.asarray(bv)[rows]
            bv_aug[base + HD] = 1.0
        in_maps.append({
            "xt": xt[b],
            "wq": np.ascontiguousarray(np.asarray(Wq)[hs, :].T),
            "wk": np.ascontiguousarray(np.asarray(Wk)[hs, :].T),
            "wv": wv_aug,
            "wo": np.ascontiguousarray(np.asarray(Wo)[:, hs].T),
            "bq": np.ascontiguousarray(np.asarray(bq)[hs].reshape(2, 128).T),
            "bk": np.ascontiguousarray(np.asarray(bk)[hs].reshape(2, 128).T),
            "bv": bv_aug,
            "mp": mp,
        })

    res = run_bass_kernel_spmd(nc, in_maps, core_ids=list(range(NCORES))).results
    out = np.empty((B, S, D), dtype=np.float32)
    for b in range(B):
        acc = res[b * HPC]["out"].astype(np.float32).copy()
        for g in range(1, HPC):
            acc += res[b * HPC + g]["out"]
        out[b] = acc + np.asarray(bo, dtype=np.float32)[None, :]
    return out
